# revision 1
# baseline (speedup 1.0000x reference)
"""Trainium2 Bass kernel for MFVIConstituency mean-field iterations.

Per batch b (one NeuronCore each, 8 total):
    q = s_con;  repeat 3x:  q[i,j] = s_con[i,j] + sum_k sig(q)[j,k] * sb[i,j,k]
    out = sigmoid(q)
where sb = s_bin * mask2o, mask2o[i,j,k] = mask[i,j] & (i!=k) & (j!=k).

Host (numpy) does: masking, fp16 cast, SBUF-cache layout packing, iteration-1
sigmoid, final transpose. Device does, per iteration: fp16 tensor_tensor mul
(DVE 2x mode) -> segmented reduction split between a DVE in-place pairwise
tree (fp16 adds at 2x) and ACT activation-accumulate, then sigmoid + xbar
transposes to rebuild the sig operand layout.

On-chip layout: q is assembled transposed (QT[j,i]); j lives on partitions in
two chunks: chunk1 = j 0:128, chunk2 "packed" = j 128:192 duplicated across
both partition halves with the i-range split (p<64: i 0:96, p>=64: i 96:192)
so every DVE instruction uses all 128 partitions.
"""

import numpy as np

S = 192
B = 8
P = 128
G = 48          # i-values per slab -> slab free size G*S = 9216
NSLAB1 = 4      # chunk1: 4 slabs of 48 i-values (j in 0:128)
NSLAB2 = 2      # chunk2 packed: 96 i-per-half * 2 halves / 48
DVE_SEGS = 34   # per slab: segments reduced by the DVE tree; rest go to ACT
SLAB_ORDER = [4, 5, 0, 1, 2, 3]   # chunk2 first so its boundary work overlaps

_CACHE = {}


def _build_program():
    import concourse.tile as tile
    from concourse import mybir, bacc
    from contextlib import ExitStack

    f32, f16 = mybir.dt.float32, mybir.dt.float16
    SLAB = G * S
    Sig = None

    nc = bacc.Bacc("TRN2", target_bir_lowering=False, debug=False, num_devices=B)
    Sig = __import__("concourse.mybir", fromlist=["x"]).ActivationFunctionType.Sigmoid
    Cpy = __import__("concourse.mybir", fromlist=["x"]).ActivationFunctionType.Copy
    c1_d = nc.dram_tensor("c1", [P, NSLAB1 * SLAB], f16, kind="ExternalInput")
    c2_d = nc.dram_tensor("c2", [P, NSLAB2 * SLAB], f16, kind="ExternalInput")
    siga_d = nc.dram_tensor("siga", [P, S], f16, kind="ExternalInput")
    sigb_d = nc.dram_tensor("sigb", [P, S], f16, kind="ExternalInput")
    sconT1_d = nc.dram_tensor("sconT1", [P, S], f32, kind="ExternalInput")
    sconT2p_d = nc.dram_tensor("sconT2p", [P, 96], f32, kind="ExternalInput")
    qt_d = nc.dram_tensor("qt_out", [S, S], f32, kind="ExternalOutput")

    with tile.TileContext(nc) as tc, ExitStack() as ctx:
        cache_p = ctx.enter_context(tc.tile_pool(name="cache", bufs=1))
        small_p = ctx.enter_context(tc.tile_pool(name="small", bufs=1))
        sig_p = ctx.enter_context(tc.tile_pool(name="sig", bufs=2))
        qt_p = ctx.enter_context(tc.tile_pool(name="qt", bufs=2))
        p_p = ctx.enter_context(tc.tile_pool(name="prod", bufs=4))
        junk_p = ctx.enter_context(tc.tile_pool(name="junk", bufs=4))
        sq_p = ctx.enter_context(tc.tile_pool(name="sq", bufs=2))
        out_p = ctx.enter_context(tc.tile_pool(name="out", bufs=1))

        sconT1_t = small_p.tile([P, S], f32, tag="sc1")
        nc.scalar.dma_start(sconT1_t[:], sconT1_d.ap())
        sconT2p_t = small_p.tile([P, 96], f32, tag="sc2")
        nc.scalar.dma_start(sconT2p_t[:], sconT2p_d.ap())
        siga_t = sig_p.tile([P, S], f16, tag="siga")
        nc.scalar.dma_start(siga_t[:], siga_d.ap())
        sigb_t = sig_p.tile([P, S], f16, tag="sigb")
        nc.scalar.dma_start(sigb_t[:], sigb_d.ap())

        cts = {}
        for idx, s in enumerate(SLAB_ORDER):
            ct = cache_p.tile([P, SLAB], f16, tag=f"c{s}")
            if s < NSLAB1:
                src = c1_d.ap()[:, s * SLAB:(s + 1) * SLAB]
            else:
                src = c2_d.ap()[:, (s - NSLAB1) * SLAB:(s - NSLAB1 + 1) * SLAB]
            eng = nc.sync
            if idx < 2:
                # split first-wave loads so compute ramps sooner
                h = SLAB // 2
                eng.dma_start(ct[:, 0:h], src[:, 0:h])
                eng.dma_start(ct[:, h:SLAB], src[:, h:SLAB])
            else:
                eng.dma_start(ct[:], src)
            cts[s] = ct

        def do_slab(s, siga_t, sigb_t, qt1, qt2, split=None):
            is1 = s < NSLAB1
            sig_t = siga_t if is1 else sigb_t
            qt_t = qt1 if is1 else qt2
            base = (s if is1 else s - NSLAB1) * G
            pt = p_p.tile([P, SLAB], f16)
            p3 = pt[:].rearrange("p (g k) -> p g k", k=S)
            in0 = cts[s][:].rearrange("p (g k) -> p g k", k=S)
            in1 = sig_t[:].unsqueeze(1).broadcast_to([P, G, S])
            if split == "g":       # ramp: match the halved first-wave DMAs
                h = G // 2
                nc.vector.tensor_tensor(p3[:, 0:h, :], in0[:, 0:h, :],
                                        in1[:, 0:h, :], mybir.AluOpType.mult)
                nc.vector.tensor_tensor(p3[:, h:G, :], in0[:, h:G, :],
                                        in1[:, h:G, :], mybir.AluOpType.mult)
            elif split == "k":     # boundary: high k-columns are ready first
                nc.vector.tensor_tensor(p3[:, :, 128:S], in0[:, :, 128:S],
                                        in1[:, :, 128:S], mybir.AluOpType.mult)
                nc.vector.tensor_tensor(p3[:, :, 0:128], in0[:, :, 0:128],
                                        in1[:, :, 0:128], mybir.AluOpType.mult)
            else:
                nc.vector.tensor_tensor(p3, in0, in1, mybir.AluOpType.mult)
            d = DVE_SEGS
            if d > 0:
                w = S
                while w > 3:   # in-place fp16 pairwise tree: 192->96->...->3
                    h = w // 2
                    nc.vector.tensor_tensor(
                        p3[:, 0:d, 0:h], p3[:, 0:d, 0:h], p3[:, 0:d, h:w],
                        mybir.AluOpType.add)
                    w = h
                nc.vector.tensor_reduce(
                    qt_t[:, base:base + d], p3[:, 0:d, 0:3],
                    axis=mybir.AxisListType.X, op=mybir.AluOpType.add)
            for g in range(d, G):
                jt = junk_p.tile([P, S], f16)
                nc.scalar.activation(
                    jt[:], pt[:, g * S:(g + 1) * S], Cpy,
                    accum_out=qt_t[:, base + g:base + g + 1])

        for it in range(3):
            qt1 = qt_p.tile([P, S], f32, tag="qt1")
            qt2 = qt_p.tile([P, 96], f32, tag="qt2")
            last = it == 2
            if not last:
                nsa = sig_p.tile([P, S], f16, tag="siga")
                nsb = sig_p.tile([P, S], f16, tag="sigb")
                sq1 = sq_p.tile([P, 256], f16, tag="sq1")
                sq2 = sq_p.tile([P, 128], f16, tag="sq2")
                tmp1 = sq_p.tile([P, 128], f16, tag="tmp1")
                tmp2 = sq_p.tile([P, 128], f16, tag="tmp2")

            for si, s in enumerate(SLAB_ORDER[0:2]):   # chunk2 slabs first
                sp = "g" if it == 0 else ("k" if si == 0 else None)
                do_slab(s, siga_t, sigb_t, qt1, qt2, split=sp)
            nc.vector.tensor_tensor(qt2[:], qt2[:], sconT2p_t[:], mybir.AluOpType.add)
            if not last:
                # chunk2 boundary work overlaps chunk1 compute below
                nc.scalar.activation(sq2[:, 0:96], qt2[:], Sig)
                nc.scalar.activation(sq2[:, 96:128], qt2[:, 0:32], Sig)  # filler
                nc.sync.dma_start_transpose(tmp2[:], sq2[:])
                nc.scalar.dma_start(nsa[0:96, 128:192], tmp2[0:96, 0:64])
                nc.scalar.dma_start(nsa[96:128, 128:192], tmp2[0:32, 64:128])
                nc.scalar.dma_start(nsb[0:64, 128:192], tmp2[32:96, 64:128])
                nc.scalar.dma_start(nsb[64:128, 128:192], tmp2[32:96, 64:128])
            else:
                o2 = out_p.tile([P, 96], f32, tag="o2")
                nc.scalar.activation(o2[:], qt2[:], Sig)
                nc.sync.dma_start(qt_d.ap()[128:192, 0:96], o2[0:64, :])
                nc.sync.dma_start(qt_d.ap()[128:192, 96:192], o2[64:128, :])

            for s in SLAB_ORDER[2:]:            # chunk1 slabs
                do_slab(s, siga_t, sigb_t, qt1, qt2)
            nc.vector.tensor_tensor(qt1[:], qt1[:], sconT1_t[:], mybir.AluOpType.add)
            if not last:
                nc.scalar.activation(sq1[:, 0:S], qt1[:], Sig)
                nc.scalar.activation(sq1[:, S:256], qt1[:, 0:64], Sig)  # filler
                nc.sync.dma_start_transpose(nsa[0:128, 0:128], sq1[:, 0:128])
                nc.sync.dma_start_transpose(tmp1[:], sq1[:, 128:256])
                nc.scalar.dma_start(nsb[0:64, 0:128], tmp1[0:64, :])
                nc.scalar.dma_start(nsb[64:128, 0:128], tmp1[0:64, :])
                siga_t, sigb_t = nsa, nsb
            else:
                o1 = out_p.tile([P, S], f32, tag="o1")
                nc.scalar.activation(o1[:], qt1[:], Sig)
                nc.sync.dma_start(qt_d.ap()[0:128, :], o1[:])
    nc.compile()
    return nc


def _get_program():
    if "nc" not in _CACHE:
        _CACHE["nc"] = _build_program()
    return _CACHE["nc"]


def _prep_core_inputs(s_con_b, sbm16_b):
    """Per-batch input dict. sbm16_b: masked s_bin, fp16, [i, j, k]."""
    A = sbm16_b
    c1 = np.ascontiguousarray(A[:, 0:128, :].transpose(1, 0, 2)).reshape(P, S * S)
    c2a = A[0:96, 128:192, :].transpose(1, 0, 2)     # [64, 96, 192]
    c2b = A[96:192, 128:192, :].transpose(1, 0, 2)   # [64, 96, 192]
    c2 = np.ascontiguousarray(np.concatenate([c2a, c2b], 0)).reshape(P, 96 * S)
    sig1 = (1.0 / (1.0 + np.exp(-s_con_b))).astype(np.float16)   # [a, k] natural
    siga = np.ascontiguousarray(sig1[0:128])
    sigb = np.ascontiguousarray(np.concatenate([sig1[128:192]] * 2, 0))
    sconT = np.ascontiguousarray(s_con_b.T)          # [j, i]
    sconT1 = sconT[0:128].copy()
    sconT2p = np.concatenate([sconT[128:192, 0:96], sconT[128:192, 96:192]], 0).copy()
    return {"c1": c1, "c2": c2, "siga": siga, "sigb": sigb,
            "sconT1": sconT1, "sconT2p": sconT2p}


def kernel(s_con, s_bin, mask):
    from concourse.bass_utils import run_bass_kernel_spmd

    s_con = np.asarray(s_con, dtype=np.float32)
    s_bin = np.asarray(s_bin, dtype=np.float32)
    mask = np.asarray(mask)

    idx = np.arange(S)
    ne = idx[:, None] != idx[None, :]                       # [a, k]
    m2 = ne[:, None, :] & ne[None, :, :]                    # [i, j, k]
    full_mask = mask[:, :, :, None] & m2[None]              # [B, i, j, k]
    sbm16 = (s_bin * full_mask).astype(np.float16)

    nc = _get_program()
    in_maps = [_prep_core_inputs(s_con[b], sbm16[b]) for b in range(B)]
    res = run_bass_kernel_spmd(nc, in_maps, list(range(B)))
    out = np.stack([res.results[b]["qt_out"].T for b in range(B)], 0)
    return np.ascontiguousarray(out.astype(np.float32))



# revision 6
# speedup vs baseline: 2.6542x; 2.6542x over previous
"""Trainium2 Bass kernel for MFVIConstituency mean-field iterations.

Per batch b (one NeuronCore each, 8 total):
    q = s_con;  repeat 3x:  q[i,j] = s_con[i,j] + sum_k sig(q)[j,k] * sb[i,j,k]
    out = sigmoid(q)
where sb = s_bin * mask2o, mask2o[i,j,k] = mask[i,j] & (i!=k) & (j!=k).

Formulation: the contraction is a batch of 192 per-j matvecs
    q[:, j] = SB_j @ sig(q)[j, :],   SB_j = sb[:, j, :]  (192x192)
mapped onto the TensorEngine: for each output column j, the stationary
operand is sb[k, i; j] (split into k-tiles 128+64 and i-tiles 128+64) and
the moving operand is the single column sig(q)^T[:, j]; the 4 matmuls
accumulate q[:, j] in PSUM (fp32).  s_con is accumulated afterwards with
two identity-stationary matmuls (out[m,c] += s_con[m,c]).  The iteration
boundary is: ACT sigmoid (PSUM->SBUF, fp16) -> PE transposes (4 blocks,
building sig(q)^T for the next iteration's moving columns) -> DVE/ACT
copies (PSUM->SBUF).  Output leaves in natural [i, j] layout.

s_bin is cached in SBUF as fp16 in [k, (j, i)] layout (14.2 MB), loaded
once in j-chunks so iteration-1 matmuls stream behind the DMA.
"""

import numpy as np

S = 192
B = 8
P = 128
K2 = 64          # second k-tile rows (k 128:192)
NCH = 8          # W load chunks
CJ = S // NCH    # j per chunk

_CACHE = {}


def _build_program():
    import concourse.tile as tile
    from concourse import mybir, bacc
    from contextlib import ExitStack

    f32, f16 = mybir.dt.float32, mybir.dt.float16
    Sig = mybir.ActivationFunctionType.Sigmoid
    Cpy = mybir.ActivationFunctionType.Copy
    JW = S * S   # 36864 columns of the W tiles

    nc = bacc.Bacc("TRN2", target_bir_lowering=False, debug=False, num_devices=B)
    w1_d = nc.dram_tensor("w1", [P, JW], f16, kind="ExternalInput")
    w2_d = nc.dram_tensor("w2", [K2, JW], f16, kind="ExternalInput")
    scon1_d = nc.dram_tensor("scon1", [P, S], f16, kind="ExternalInput")
    scon2_d = nc.dram_tensor("scon2", [K2, S], f16, kind="ExternalInput")
    r01_d = nc.dram_tensor("r01", [P, S], f16, kind="ExternalInput")
    r02_d = nc.dram_tensor("r02", [K2, S], f16, kind="ExternalInput")
    ident_d = nc.dram_tensor("ident", [P, P], f16, kind="ExternalInput")
    q_d = nc.dram_tensor("q_out", [S, S], f32, kind="ExternalOutput")

    with tile.TileContext(nc) as tc, ExitStack() as ctx:
        w_p = ctx.enter_context(tc.tile_pool(name="w", bufs=1))
        c_p = ctx.enter_context(tc.tile_pool(name="const", bufs=1))
        r_p = ctx.enter_context(tc.tile_pool(name="r", bufs=2))
        x_p = ctx.enter_context(tc.tile_pool(name="x", bufs=2))
        o_p = ctx.enter_context(tc.tile_pool(name="o", bufs=1))
        qa_p = ctx.enter_context(tc.tile_pool(name="qa", bufs=2, space="PSUM"))
        qb_p = ctx.enter_context(tc.tile_pool(name="qb", bufs=2, space="PSUM"))
        t_p = ctx.enter_context(tc.tile_pool(name="t", bufs=1, space="PSUM"))

        # constants / first-iteration sigmoid operand (host-computed)
        ident_t = c_p.tile([P, P], f16, tag="ident")
        nc.scalar.dma_start(ident_t[:], ident_d.ap())
        scon1_t = c_p.tile([P, S], f16, tag="scon1")
        nc.scalar.dma_start(scon1_t[:], scon1_d.ap())
        scon2_t = c_p.tile([K2, S], f16, tag="scon2")
        nc.scalar.dma_start(scon2_t[:], scon2_d.ap())
        r1_t = r_p.tile([P, S], f16, tag="r1")
        nc.scalar.dma_start(r1_t[:], r01_d.ap())
        r2_t = r_p.tile([K2, S], f16, tag="r2")
        nc.scalar.dma_start(r2_t[:], r02_d.ap())

        # s_bin cache, streamed in j-chunks (w1 chunk then w2 chunk so the
        # first iteration's column-j matmuls can start as soon as chunk 0
        # lands)
        w1_t = w_p.tile([P, JW], f16, tag="w1")
        w2_t = w_p.tile([K2, JW], f16, tag="w2")
        CW = CJ * S
        for c in range(NCH):
            sl = slice(c * CW, (c + 1) * CW)
            nc.sync.dma_start(w1_t[:, sl], w1_d.ap()[:, sl])
            nc.sync.dma_start(w2_t[:, sl], w2_d.ap()[:, sl])

        for it in range(3):
            qa = qa_p.tile([P, S], f32, tag="qa")
            qb = qb_p.tile([K2, S], f32, tag="qb")
            # q = s_con first, via identity-stationary matmuls
            # (out[m,c] = rhs[m,c]).  start=True sets has_written for the
            # whole tile, so every later column matmul accumulates; a
            # per-column start=True would clear has_written BANK-wide and
            # break accumulation for the other columns.
            nc.tensor.matmul(qa[:], ident_t[:], scon1_t[:],
                             start=True, stop=False, skip_group_check=True)
            nc.tensor.matmul(qb[:], ident_t[0:K2, 0:K2], scon2_t[:],
                             start=True, stop=False, skip_group_check=True)
            for j in range(S):
                base = j * S
                last = j == S - 1
                rj1 = r1_t[:, j:j + 1]
                rj2 = r2_t[:, j:j + 1]
                nc.tensor.matmul(qa[:, j:j + 1], w1_t[:, base:base + P], rj1,
                                 start=False, stop=False, skip_group_check=True)
                nc.tensor.matmul(qa[:, j:j + 1], w2_t[:, base:base + P], rj2,
                                 start=False, stop=last, skip_group_check=True)
                nc.tensor.matmul(qb[:, j:j + 1], w1_t[:, base + P:base + S], rj1,
                                 start=False, stop=False, skip_group_check=True)
                nc.tensor.matmul(qb[:, j:j + 1], w2_t[:, base + P:base + S], rj2,
                                 start=False, stop=last, skip_group_check=True)

            if it < 2:
                # boundary: X = sigmoid(q) (fp16), R = X^T for next iteration
                x1 = x_p.tile([P, S], f16, tag="x1")
                nc.scalar.activation(x1[:], qa[:], Sig)
                x2 = x_p.tile([K2, S], f16, tag="x2")
                nc.scalar.activation(x2[:], qb[:], Sig)
                # all 4 transpose blocks packed into one PSUM bank
                tt = t_p.tile([P, 3 * P], f16, tag="tt")
                t1 = tt[:, 0:P]              # X[j 0:128, k 0:128]^T
                t2 = tt[0:K2, P:2 * P]       # X[j 0:128, k 128:192]^T
                t3 = tt[:, 2 * P:2 * P + K2]     # X[j 128:192, k 0:128]^T
                t4 = tt[0:K2, 2 * P + K2:3 * P]  # X[j 128:192, k 128:192]^T
                nc.tensor.transpose(t1, x1[:, 0:P], ident_t[:])
                nc.tensor.transpose(t2, x1[:, P:S], ident_t[:])
                nc.tensor.transpose(t3, x2[:, 0:P], ident_t[0:K2, 0:K2])
                nc.tensor.transpose(t4, x2[:, P:S], ident_t[0:K2, 0:K2])
                r1_t = r_p.tile([P, S], f16, tag="r1")
                r2_t = r_p.tile([K2, S], f16, tag="r2")
                nc.vector.tensor_copy(r1_t[:, 0:P], t1)
                nc.scalar.activation(r2_t[:, 0:P], t2, Cpy)
                nc.vector.tensor_copy(r1_t[:, P:S], t3)
                nc.scalar.activation(r2_t[:, P:S], t4, Cpy)
            else:
                o1 = o_p.tile([P, S], f32, tag="o1")
                nc.scalar.activation(o1[:], qa[:], Sig)
                nc.sync.dma_start(q_d.ap()[0:P, :], o1[:])
                o2 = o_p.tile([K2, S], f32, tag="o2")
                nc.scalar.activation(o2[:], qb[:], Sig)
                nc.sync.dma_start(q_d.ap()[P:S, :], o2[:])
    nc.compile()
    return nc


def _get_program():
    if "nc" not in _CACHE:
        _CACHE["nc"] = _build_program()
    return _CACHE["nc"]


def _prep_core_inputs(s_con_b, sbm16_b, ident):
    """Per-batch input dict. sbm16_b: masked s_bin, fp16, [i, j, k]."""
    kt = np.ascontiguousarray(sbm16_b.transpose(2, 1, 0))   # [k, j, i]
    w1 = kt[0:P].reshape(P, S * S)
    w2 = kt[P:S].reshape(K2, S * S)
    sig0 = (1.0 / (1.0 + np.exp(-s_con_b.astype(np.float64)))).astype(np.float16)
    r01 = np.ascontiguousarray(sig0[:, 0:P].T)               # [k 0:128, j]
    r02 = np.ascontiguousarray(sig0[:, P:S].T)               # [k 128:192, j]
    sc16 = s_con_b.astype(np.float16)
    return {"w1": w1, "w2": w2,
            "scon1": np.ascontiguousarray(sc16[0:P]),
            "scon2": np.ascontiguousarray(sc16[P:S]),
            "r01": r01, "r02": r02, "ident": ident}


def kernel(s_con, s_bin, mask):
    from concourse.bass_utils import run_bass_kernel_spmd

    s_con = np.asarray(s_con, dtype=np.float32)
    s_bin = np.asarray(s_bin, dtype=np.float32)
    mask = np.asarray(mask)

    idx = np.arange(S)
    ne = idx[:, None] != idx[None, :]                       # [a, k]
    m2 = ne[:, None, :] & ne[None, :, :]                    # [i, j, k]
    full_mask = mask[:, :, :, None] & m2[None]              # [B, i, j, k]
    sbm16 = (s_bin * full_mask).astype(np.float16)

    ident = np.eye(P, dtype=np.float16)
    nc = _get_program()
    in_maps = [_prep_core_inputs(s_con[b], sbm16[b], ident) for b in range(B)]
    res = run_bass_kernel_spmd(nc, in_maps, list(range(B)))
    out = np.stack([res.results[b]["q_out"] for b in range(B)], 0)
    return np.ascontiguousarray(out.astype(np.float32))


# revision 8
# speedup vs baseline: 4.3049x; 1.6220x over previous
"""Trainium2 Bass kernel for MFVIConstituency mean-field iterations.

Per batch b (one NeuronCore each, 8 total):
    q = s_con;  repeat 3x:  q[i,j] = s_con[i,j] + sum_k sig(q)[j,k] * sb[i,j,k]
    out = sigmoid(q)
where sb = s_bin * mask2o, mask2o[i,j,k] = mask[i,j] & (i!=k) & (j!=k).

Formulation: the contraction is a batch of 192 per-j matvecs
    q[:, j] = SB_j @ sig(q)[j, :],   SB_j = sb[:, j, :]  (192x192)
mapped onto the TensorEngine: for each output column j the stationary
operand is sb[k, i; j] (k-tiles 128+64, i-tiles 128+64) and the moving
operand is the single column sig(q)^T[:, j]; 4 matmuls accumulate
q[:, j] in PSUM (fp32).  s_con enters first through identity-stationary
matmuls (start=True sets has_written for the whole tile; a per-column
start would clear has_written BANK-wide and break accumulation).

Boundary between iterations builds X = sigmoid(q) (fp16) and R = X^T
(the next iteration's moving columns), split by column halves so the
left half (q cols 0:128) runs while the right-half matmuls / DMA are
still in flight: ACT sigmoid (PSUM->SBUF) -> PE transpose blocks ->
DVE copies (PSUM->SBUF).  Output leaves in natural [i, j] layout.

s_bin is cached in SBUF as fp16 in [k, (j, i)] layout (14.2 MB), loaded
once in 24-column groups striped over all three DMA-capable queues
(SP / Activation / GpSimd) so the transfers overlap three-way and
arrive in j order; iteration-1 matmuls stream right behind them.
"""

import numpy as np

S = 192
B = 8
P = 128
K2 = 64          # second k-tile rows (k 128:192), also lower half of i
NG = 8           # W load groups
CJ = S // NG     # j per group

_CACHE = {}


def _build_program():
    import concourse.tile as tile
    from concourse import mybir, bacc
    from contextlib import ExitStack

    f32, f16 = mybir.dt.float32, mybir.dt.float16
    Sig = mybir.ActivationFunctionType.Sigmoid
    JW = S * S   # 36864 columns of the W tiles

    nc = bacc.Bacc("TRN2", target_bir_lowering=False, debug=False, num_devices=B)
    w1_d = nc.dram_tensor("w1", [P, JW], f16, kind="ExternalInput")
    w2_d = nc.dram_tensor("w2", [K2, JW], f16, kind="ExternalInput")
    scon1_d = nc.dram_tensor("scon1", [P, S], f16, kind="ExternalInput")
    scon2_d = nc.dram_tensor("scon2", [K2, S], f16, kind="ExternalInput")
    r01_d = nc.dram_tensor("r01", [P, S], f16, kind="ExternalInput")
    r02_d = nc.dram_tensor("r02", [K2, S], f16, kind="ExternalInput")
    ident_d = nc.dram_tensor("ident", [P, P], f16, kind="ExternalInput")
    q_d = nc.dram_tensor("q_out", [S, S], f32, kind="ExternalOutput")

    with tile.TileContext(nc) as tc, ExitStack() as ctx:
        w_p = ctx.enter_context(tc.tile_pool(name="w", bufs=1))
        c_p = ctx.enter_context(tc.tile_pool(name="const", bufs=1))
        r_p = ctx.enter_context(tc.tile_pool(name="r", bufs=2))
        x_p = ctx.enter_context(tc.tile_pool(name="x", bufs=2))
        o_p = ctx.enter_context(tc.tile_pool(name="o", bufs=1))
        qa_p = ctx.enter_context(tc.tile_pool(name="qa", bufs=2, space="PSUM"))
        qb_p = ctx.enter_context(tc.tile_pool(name="qb", bufs=2, space="PSUM"))
        t_p = ctx.enter_context(tc.tile_pool(name="t", bufs=1, space="PSUM"))

        # constants / first-iteration sigmoid operand (host-computed)
        ident_t = c_p.tile([P, P], f16, tag="ident")
        nc.sync.dma_start(ident_t[:], ident_d.ap())
        r1_t = r_p.tile([P, S], f16, tag="r1")
        nc.sync.dma_start(r1_t[:], r01_d.ap())
        r2_t = r_p.tile([K2, S], f16, tag="r2")
        nc.sync.dma_start(r2_t[:], r02_d.ap())
        scon1_t = c_p.tile([P, S], f16, tag="scon1")
        nc.scalar.dma_start(scon1_t[:], scon1_d.ap())
        scon2_t = c_p.tile([K2, S], f16, tag="scon2")
        nc.scalar.dma_start(scon2_t[:], scon2_d.ap())

        # s_bin cache: per 24-column group, three equal 0.59MB transfers on
        # the three DMA queues (w1 upper/lower partition halves + w2), so
        # the groups arrive j-ordered with 3-way transfer overlap.
        w1_t = w_p.tile([P, JW], f16, tag="w1")
        w2_t = w_p.tile([K2, JW], f16, tag="w2")
        CW = CJ * S
        for g in range(NG):
            sl = slice(g * CW, (g + 1) * CW)
            nc.sync.dma_start(w1_t[0:K2, sl], w1_d.ap()[0:K2, sl])
            nc.scalar.dma_start(w1_t[K2:P, sl], w1_d.ap()[K2:P, sl])
            nc.gpsimd.dma_start(w2_t[:, sl], w2_d.ap()[:, sl])

        def col_matmuls(qa, qb, r1_t, r2_t, j0, j1):
            for j in range(j0, j1):
                base = j * S
                last = j == S - 1
                rj1 = r1_t[:, j:j + 1]
                rj2 = r2_t[:, j:j + 1]
                nc.tensor.matmul(qa[:, j:j + 1], w1_t[:, base:base + P], rj1,
                                 start=False, stop=False, skip_group_check=True)
                nc.tensor.matmul(qa[:, j:j + 1], w2_t[:, base:base + P], rj2,
                                 start=False, stop=last, skip_group_check=True)
                nc.tensor.matmul(qb[:, j:j + 1], w1_t[:, base + P:base + S], rj1,
                                 start=False, stop=False, skip_group_check=True)
                nc.tensor.matmul(qb[:, j:j + 1], w2_t[:, base + P:base + S], rj2,
                                 start=False, stop=last, skip_group_check=True)

        for it in range(3):
            qa = qa_p.tile([P, S], f32, tag="qa")
            qb = qb_p.tile([K2, S], f32, tag="qb")
            # q = s_con first (identity stationary: out[m,c] = rhs[m,c])
            nc.tensor.matmul(qa[:], ident_t[:], scon1_t[:],
                             start=True, stop=False, skip_group_check=True)
            nc.tensor.matmul(qb[:], ident_t[0:K2, 0:K2], scon2_t[:],
                             start=True, stop=False, skip_group_check=True)
            col_matmuls(qa, qb, r1_t, r2_t, 0, S)

            if it < 2:
                # left half (q cols 0:128) overlaps the right-half tail
                x1 = x_p.tile([P, S], f16, tag="x1")
                x2 = x_p.tile([K2, S], f16, tag="x2")
                nc.scalar.activation(x1[:, 0:P], qa[:, 0:P], Sig)
                nc.scalar.activation(x2[:, 0:P], qb[:, 0:P], Sig)
                nc.scalar.activation(x1[:, P:S], qa[:, P:S], Sig)
                nc.scalar.activation(x2[:, P:S], qb[:, P:S], Sig)
                tt = t_p.tile([P, 3 * P], f16, tag="tt")
                t1 = tt[:, 0:P]                    # [128,128] X[r 0:128, c 0:128]^T
                t3 = tt[:, P:P + K2]               # [128, 64] X[r 128:192, c 0:128]^T
                t2 = tt[0:K2, P + K2:2 * P + K2]   # [ 64,128] X[r 0:128, c 128:192]^T
                t4 = tt[0:K2, 2 * P + K2:3 * P]    # [ 64, 64] X[r 128:192, c 128:]^T
                nc.tensor.transpose(t1, x1[:, 0:P], ident_t[:])
                nc.tensor.transpose(t3, x2[:, 0:P], ident_t[0:K2, 0:K2])
                nc.tensor.transpose(t2, x1[:, P:S], ident_t[:])
                nc.tensor.transpose(t4, x2[:, P:S], ident_t[0:K2, 0:K2])
                r1_t = r_p.tile([P, S], f16, tag="r1")
                r2_t = r_p.tile([K2, S], f16, tag="r2")
                nc.vector.tensor_copy(r1_t[:, 0:P], t1)
                nc.vector.tensor_copy(r1_t[:, P:S], t3)
                nc.vector.tensor_copy(r2_t[:, 0:P], t2)
                nc.vector.tensor_copy(r2_t[:, P:S], t4)
            else:
                o1 = o_p.tile([P, S], f32, tag="o1")
                o2 = o_p.tile([K2, S], f32, tag="o2")
                nc.scalar.activation(o1[:, 0:P], qa[:, 0:P], Sig)
                nc.sync.dma_start(q_d.ap()[0:P, 0:P], o1[:, 0:P])
                nc.scalar.activation(o2[:, 0:P], qb[:, 0:P], Sig)
                nc.scalar.dma_start(q_d.ap()[P:S, 0:P], o2[:, 0:P])
                nc.scalar.activation(o1[:, P:S], qa[:, P:S], Sig)
                nc.sync.dma_start(q_d.ap()[0:P, P:S], o1[:, P:S])
                nc.scalar.activation(o2[:, P:S], qb[:, P:S], Sig)
                nc.scalar.dma_start(q_d.ap()[P:S, P:S], o2[:, P:S])
    nc.compile()
    return nc


def _get_program():
    if "nc" not in _CACHE:
        _CACHE["nc"] = _build_program()
    return _CACHE["nc"]


def _prep_core_inputs(s_con_b, sbm16_b, ident):
    """Per-batch input dict. sbm16_b: masked s_bin, fp16, [i, j, k]."""
    kt = np.ascontiguousarray(sbm16_b.transpose(2, 1, 0))   # [k, j, i]
    w1 = kt[0:P].reshape(P, S * S)
    w2 = kt[P:S].reshape(K2, S * S)
    sig0 = (1.0 / (1.0 + np.exp(-s_con_b.astype(np.float64)))).astype(np.float16)
    r01 = np.ascontiguousarray(sig0[:, 0:P].T)               # [k 0:128, j]
    r02 = np.ascontiguousarray(sig0[:, P:S].T)               # [k 128:192, j]
    sc16 = s_con_b.astype(np.float16)
    return {"w1": w1, "w2": w2,
            "scon1": np.ascontiguousarray(sc16[0:P]),
            "scon2": np.ascontiguousarray(sc16[P:S]),
            "r01": r01, "r02": r02, "ident": ident}


def kernel(s_con, s_bin, mask):
    from concourse.bass_utils import run_bass_kernel_spmd

    s_con = np.asarray(s_con, dtype=np.float32)
    s_bin = np.asarray(s_bin, dtype=np.float32)
    mask = np.asarray(mask)

    idx = np.arange(S)
    ne = idx[:, None] != idx[None, :]                       # [a, k]
    m2 = ne[:, None, :] & ne[None, :, :]                    # [i, j, k]
    full_mask = mask[:, :, :, None] & m2[None]              # [B, i, j, k]
    sbm16 = (s_bin * full_mask).astype(np.float16)

    ident = np.eye(P, dtype=np.float16)
    nc = _get_program()
    in_maps = [_prep_core_inputs(s_con[b], sbm16[b], ident) for b in range(B)]
    res = run_bass_kernel_spmd(nc, in_maps, list(range(B)))
    out = np.stack([res.results[b]["q_out"] for b in range(B)], 0)
    return np.ascontiguousarray(out.astype(np.float32))


# revision 10
# speedup vs baseline: 7.0156x; 1.6297x over previous
"""Trainium2 Bass kernel for MFVIConstituency mean-field iterations.

Per batch b (one NeuronCore each, 8 total):
    q = s_con;  repeat 3x:  q[i,j] = s_con[i,j] + sum_k sig(q)[j,k] * sb[i,j,k]
    out = sigmoid(q)
where sb = s_bin * mask2o, mask2o[i,j,k] = mask[i,j] & (i!=k) & (j!=k).

Formulation: the contraction is a batch of 192 per-j matvecs
    q[:, j] = SB_j @ sig(q)[j, :],   SB_j = sb[:, j, :]  (192x192)
mapped onto the TensorEngine: for each output column j the stationary
operand is sb[k, i; j] (k-tiles 128+64, i-tiles 128+64) and the moving
operand is the single column sig(q)^T[:, j]; 4 matmuls accumulate
q[:, j] in PSUM (fp32).  s_con enters first through identity-stationary
matmuls (start=True sets has_written for the whole tile; a per-column
start would clear has_written BANK-wide and break accumulation).

s_bin lives in SBUF as fp16.  The DMA cost model charges free-dim bytes
per partition (partition count is free), so everything is packed into
128 partitions: the 64-row k-tile-2 blocks ride the upper partition
half (two j-blocks sharing 128 partitions), and the whole 14.2MB cache
is striped over the three DMA queues (SP / Activation / GpSimd) in
8-column j-blocks, round-robin, so the three transfers overlap and
columns arrive in j order; iteration-1 matmuls stream right behind.
Because lhsT and rhs must share a base partition, sig(q)^T k-rows
128:192 are kept duplicated on partitions 0:64 AND 64:128 (the PE
transposes write each block twice).

Boundary between iterations builds X = sigmoid(q) (fp16) and R = X^T,
split by column halves so the left half (q cols 0:128) runs while the
right-half matmuls / DMA are still in flight: ACT sigmoid (PSUM->SBUF)
-> PE transpose blocks -> DVE/ACT copies.  Output leaves in natural
[i, j] layout.
"""

import numpy as np

S = 192
B = 8
P = 128
K2 = 64          # k-tile-2 rows (k 128:192), also lower half of i
BJ = 8           # j per block
NB = S // BJ     # 24 blocks, striped round-robin over 3 queues
BW = BJ * S      # 1536 elements per (block, k-tile)
SEG = 3 * BW     # 4608 elements per block-pair segment
QW = 4 * SEG     # 18432 elements per queue tensor

_CACHE = {}


def _wslices(j):
    """j -> (queue, w1 col base, w2 col base, w2 partition range)."""
    b, jj = divmod(j, BJ)
    q, m = b % 3, b // 3
    p, which = divmod(m, 2)
    c1 = p * SEG + which * BW + jj * S
    c2 = p * SEG + 2 * BW + jj * S
    pr = (0, K2) if which == 0 else (K2, P)
    return q, c1, c2, pr


def _build_program():
    import concourse.tile as tile
    from concourse import mybir, bacc
    from contextlib import ExitStack

    f32, f16 = mybir.dt.float32, mybir.dt.float16
    Sig = mybir.ActivationFunctionType.Sigmoid
    Cpy = mybir.ActivationFunctionType.Copy

    nc = bacc.Bacc("TRN2", target_bir_lowering=False, debug=False, num_devices=B)
    wq_d = [nc.dram_tensor(f"wq{q}", [P, QW], f16, kind="ExternalInput")
            for q in range(3)]
    scon1_d = nc.dram_tensor("scon1", [P, S], f16, kind="ExternalInput")
    scon2_d = nc.dram_tensor("scon2", [K2, S], f16, kind="ExternalInput")
    r01_d = nc.dram_tensor("r01", [P, S], f16, kind="ExternalInput")
    r02d_d = nc.dram_tensor("r02d", [P, S], f16, kind="ExternalInput")
    ident_d = nc.dram_tensor("ident", [P, P], f16, kind="ExternalInput")
    q_d = nc.dram_tensor("q_out", [S, S], f32, kind="ExternalOutput")

    with tile.TileContext(nc) as tc, ExitStack() as ctx:
        w_p = ctx.enter_context(tc.tile_pool(name="w", bufs=1))
        c_p = ctx.enter_context(tc.tile_pool(name="const", bufs=1))
        r_p = ctx.enter_context(tc.tile_pool(name="r", bufs=2))
        x_p = ctx.enter_context(tc.tile_pool(name="x", bufs=2))
        o_p = ctx.enter_context(tc.tile_pool(name="o", bufs=1))
        qa_p = ctx.enter_context(tc.tile_pool(name="qa", bufs=2, space="PSUM"))
        qb_p = ctx.enter_context(tc.tile_pool(name="qb", bufs=2, space="PSUM"))
        t_p = ctx.enter_context(tc.tile_pool(name="t", bufs=1, space="PSUM"))

        # preload the sigmoid activation table while the DMA phase runs
        jnk = c_p.tile([1, 2], f16, tag="jnk")
        nc.vector.memset(jnk[:], 0.0)
        jnk2 = c_p.tile([1, 2], f16, tag="jnk2")
        nc.scalar.activation(jnk2[:], jnk[:], Sig)

        # constants / first-iteration sigmoid operand (host-computed)
        ident_t = c_p.tile([P, P], f16, tag="ident")
        nc.sync.dma_start(ident_t[:], ident_d.ap())
        r1_t = r_p.tile([P, S], f16, tag="r1")
        nc.sync.dma_start(r1_t[:], r01_d.ap())
        # rows 0:64 and 64:128 both hold R2 = sig(s_con)^T rows 128:192
        r2_t = r_p.tile([P, S], f16, tag="r2")
        nc.sync.dma_start(r2_t[:], r02d_d.ap())
        scon1_t = c_p.tile([P, S], f16, tag="scon1")
        nc.gpsimd.dma_start(scon1_t[:], scon1_d.ap())
        scon2_t = c_p.tile([K2, S], f16, tag="scon2")
        nc.gpsimd.dma_start(scon2_t[:], scon2_d.ap())

        # s_bin cache: 3 striped queue tensors, 4 segment chunks each
        wt = [w_p.tile([P, QW], f16, tag=f"wq{q}", name=f"wq{q}") for q in range(3)]
        queues = [nc.sync, nc.scalar, nc.gpsimd]
        for p in range(4):
            sl = slice(p * SEG, (p + 1) * SEG)
            for q in range(3):
                queues[q].dma_start(wt[q][:, sl], wq_d[q].ap()[:, sl])

        def col_matmuls(qa, qb, r1_t, r2_t, j0, j1):
            for j in range(j0, j1):
                q, c1, c2, (p0, p1) = _wslices(j)
                last = j == S - 1
                t = wt[q]
                rj1 = r1_t[:, j:j + 1]
                rj2 = r2_t[p0:p1, j:j + 1]
                nc.tensor.matmul(qa[:, j:j + 1], t[:, c1:c1 + P], rj1,
                                 start=False, stop=False, skip_group_check=True)
                nc.tensor.matmul(qa[:, j:j + 1], t[p0:p1, c2:c2 + P], rj2,
                                 start=False, stop=False, skip_group_check=True)
                nc.tensor.matmul(qb[:, j:j + 1], t[:, c1 + P:c1 + S], rj1,
                                 start=False, stop=last, skip_group_check=True)
                nc.tensor.matmul(qb[:, j:j + 1], t[p0:p1, c2 + P:c2 + S], rj2,
                                 start=False, stop=last, skip_group_check=True)

        CA, CB = P + K2, 2 * P + K2   # tt column bases for t2/t4 blocks

        def boundary_left(qa, qb, x1, x2, tt, r1n):
            nc.scalar.activation(x1[:, 0:P], qa[:, 0:P], Sig)
            nc.scalar.activation(x2[:, 0:P], qb[:, 0:P], Sig)
            nc.tensor.transpose(tt[:, 0:P], x1[:, 0:P], ident_t[:])
            nc.tensor.transpose(tt[:, P:P + K2], x2[:, 0:P], ident_t[0:K2, 0:K2])
            nc.vector.tensor_copy(r1n[:, 0:P], tt[:, 0:P])
            nc.scalar.activation(r1n[:, P:S], tt[:, P:P + K2], Cpy)

        def boundary_right(qa, qb, x1, x2, tt, r2n):
            nc.scalar.activation(x1[:, P:S], qa[:, P:S], Sig)
            nc.scalar.activation(x2[:, P:S], qb[:, P:S], Sig)
            nc.tensor.transpose(tt[0:K2, CA:CA + P], x1[:, P:S], ident_t[:])
            nc.tensor.transpose(tt[K2:P, CA:CA + P], x1[:, P:S], ident_t[:])
            nc.tensor.transpose(tt[0:K2, CB:CB + K2], x2[:, P:S],
                                ident_t[0:K2, 0:K2])
            nc.tensor.transpose(tt[K2:P, CB:CB + K2], x2[:, P:S],
                                ident_t[0:K2, 0:K2])
            nc.vector.tensor_copy(r2n[0:K2, 0:P], tt[0:K2, CA:CA + P])
            nc.vector.tensor_copy(r2n[K2:P, 0:P], tt[K2:P, CA:CA + P])
            nc.scalar.activation(r2n[0:K2, P:S], tt[0:K2, CB:CB + K2], Cpy)
            nc.scalar.activation(r2n[K2:P, P:S], tt[K2:P, CB:CB + K2], Cpy)

        for it in range(3):
            qa = qa_p.tile([P, S], f32, tag="qa")
            qb = qb_p.tile([K2, S], f32, tag="qb")
            # q = s_con first (identity stationary: out[m,c] = rhs[m,c])
            nc.tensor.matmul(qa[:], ident_t[:], scon1_t[:],
                             start=True, stop=False, skip_group_check=True)
            nc.tensor.matmul(qb[:], ident_t[0:K2, 0:K2], scon2_t[:],
                             start=True, stop=False, skip_group_check=True)
            if it < 2:
                x1 = x_p.tile([P, S], f16, tag="x1")
                x2 = x_p.tile([K2, S], f16, tag="x2")
                tt = t_p.tile([P, 3 * P], f16, tag="tt")
                r1n = r_p.tile([P, S], f16, tag="r1")
                r2n = r_p.tile([P, S], f16, tag="r2")
                if it == 0:
                    # DMA-bound: left-half boundary hides under the load
                    col_matmuls(qa, qb, r1_t, r2_t, 0, P)
                    boundary_left(qa, qb, x1, x2, tt, r1n)
                    col_matmuls(qa, qb, r1_t, r2_t, P, S)
                else:
                    col_matmuls(qa, qb, r1_t, r2_t, 0, S)
                    boundary_left(qa, qb, x1, x2, tt, r1n)
                boundary_right(qa, qb, x1, x2, tt, r2n)
                r1_t, r2_t = r1n, r2n
            else:
                col_matmuls(qa, qb, r1_t, r2_t, 0, S)
                o1 = o_p.tile([P, S], f32, tag="o1")
                nc.scalar.activation(o1[:], qa[:], Sig)
                nc.sync.dma_start(q_d.ap()[0:P, :], o1[:])
                o2 = o_p.tile([K2, S], f32, tag="o2")
                nc.scalar.activation(o2[:], qb[:], Sig)
                nc.scalar.dma_start(q_d.ap()[P:S, :], o2[:])
    nc.compile()
    return nc


def _get_program():
    if "nc" not in _CACHE:
        _CACHE["nc"] = _build_program()
    return _CACHE["nc"]


def _prep_core_inputs(s_con_b, sbm16_b, ident):
    """Per-batch input dict. sbm16_b: masked s_bin, fp16, [i, j, k]."""
    kt = np.ascontiguousarray(sbm16_b.transpose(2, 1, 0))   # [k, j, i]
    w1 = kt[0:P].reshape(P, NB, BW)                          # k 0:128
    w2 = kt[P:S].reshape(K2, NB, BW)                         # k 128:192
    out = {"ident": ident}
    for q in range(3):
        bs = [q + 3 * m for m in range(NB // 3)]
        segs = []
        for p in range(4):
            b0, b1 = bs[2 * p], bs[2 * p + 1]
            segs.append(np.concatenate(
                [w1[:, b0], w1[:, b1],
                 np.concatenate([w2[:, b0], w2[:, b1]], axis=0)], axis=1))
        out[f"wq{q}"] = np.ascontiguousarray(np.concatenate(segs, axis=1))
    sig0 = (1.0 / (1.0 + np.exp(-s_con_b.astype(np.float64)))).astype(np.float16)
    out["r01"] = np.ascontiguousarray(sig0[:, 0:P].T)        # [k 0:128, j]
    r02 = sig0[:, P:S].T                                     # [k 128:192, j]
    out["r02d"] = np.ascontiguousarray(np.concatenate([r02, r02], axis=0))
    sc16 = s_con_b.astype(np.float16)
    out["scon1"] = np.ascontiguousarray(sc16[0:P])
    out["scon2"] = np.ascontiguousarray(sc16[P:S])
    return out


def kernel(s_con, s_bin, mask):
    from concourse.bass_utils import run_bass_kernel_spmd

    s_con = np.asarray(s_con, dtype=np.float32)
    s_bin = np.asarray(s_bin, dtype=np.float32)
    mask = np.asarray(mask)

    idx = np.arange(S)
    ne = idx[:, None] != idx[None, :]                       # [a, k]
    m2 = ne[:, None, :] & ne[None, :, :]                    # [i, j, k]
    full_mask = mask[:, :, :, None] & m2[None]              # [B, i, j, k]
    sbm16 = (s_bin * full_mask).astype(np.float16)

    ident = np.eye(P, dtype=np.float16)
    nc = _get_program()
    in_maps = [_prep_core_inputs(s_con[b], sbm16[b], ident) for b in range(B)]
    res = run_bass_kernel_spmd(nc, in_maps, list(range(B)))
    out = np.stack([res.results[b]["q_out"] for b in range(B)], 0)
    return np.ascontiguousarray(out.astype(np.float32))


# revision 14
# speedup vs baseline: 7.2696x; 1.0362x over previous
"""Trainium2 Bass kernel for MFVIConstituency mean-field iterations.

Per batch b (one NeuronCore each, 8 total):
    q = s_con;  repeat 3x:  q[i,j] = s_con[i,j] + sum_k sig(q)[j,k] * sb[i,j,k]
    out = sigmoid(q)
where sb = s_bin * mask2o, mask2o[i,j,k] = mask[i,j] & (i!=k) & (j!=k).

Formulation: the contraction is a batch of 192 per-j matvecs
    q[:, j] = SB_j @ sig(q)[j, :],   SB_j = sb[:, j, :]  (192x192)
mapped onto the TensorEngine: for each output column j the stationary
operand is sb[k, i; j] (k-tiles 128+64, i-tiles 128+64) and the moving
operand is the single column sig(q)^T[:, j]; 4 matmuls accumulate
q[:, j] in PSUM (fp32).  s_con enters first through identity-stationary
matmuls (start=True sets has_written for the whole tile; a per-column
start would clear has_written BANK-wide and break accumulation).

s_bin lives in SBUF as fp16.  The DMA cost model charges free-dim bytes
per partition (partition count is free), so everything is packed into
128 partitions: the 64-row k-tile-2 blocks ride the upper partition
half (two j-blocks sharing 128 partitions), and the whole 14.2MB cache
is striped over the three DMA queues (SP / Activation / GpSimd) in
8-column j-blocks, round-robin, so the three transfers overlap and
columns arrive in j order; iteration-1 matmuls stream right behind.
Because lhsT and rhs must share a base partition, sig(q)^T k-rows
128:192 are kept duplicated on partitions 0:64 AND 64:128 (the PE
transposes write each block twice).

Boundary between iterations builds X = sigmoid(q) (fp16) and R = X^T,
split by column halves so the left half (q cols 0:128) runs while the
right-half matmuls / DMA are still in flight: ACT sigmoid (PSUM->SBUF)
-> PE transpose blocks -> DVE/ACT copies.  Output leaves in natural
[i, j] layout.
"""

import numpy as np

S = 192
B = 8
P = 128
K2 = 64          # k-tile-2 rows (k 128:192), also lower half of i
BJ = 8           # j per block
NB = S // BJ     # 24 blocks, striped round-robin over 3 queues
BW = BJ * S      # 1536 elements per (block, k-tile)
SEG = 3 * BW     # 4608 elements per block-pair segment
QW = 4 * SEG     # 18432 elements per queue tensor

_CACHE = {}


def _wslices(j):
    """j -> (queue, w1 col base, w2 col base, w2 partition range)."""
    b, jj = divmod(j, BJ)
    q, m = b % 3, b // 3
    p, which = divmod(m, 2)
    c1 = p * SEG + which * BW + jj * S
    c2 = p * SEG + 2 * BW + jj * S
    pr = (0, K2) if which == 0 else (K2, P)
    return q, c1, c2, pr


def _build_program():
    import concourse.tile as tile
    from concourse import mybir, bacc
    from contextlib import ExitStack

    f32, f16 = mybir.dt.float32, mybir.dt.float16
    Sig = mybir.ActivationFunctionType.Sigmoid
    Cpy = mybir.ActivationFunctionType.Copy

    nc = bacc.Bacc("TRN2", target_bir_lowering=False, debug=False, num_devices=B)
    wq_d = [nc.dram_tensor(f"wq{q}", [P, QW], f16, kind="ExternalInput")
            for q in range(3)]
    scon1_d = nc.dram_tensor("scon1", [P, S], f16, kind="ExternalInput")
    scon2_d = nc.dram_tensor("scon2", [K2, S], f16, kind="ExternalInput")
    r01_d = nc.dram_tensor("r01", [P, S], f16, kind="ExternalInput")
    r02d_d = nc.dram_tensor("r02d", [P, S], f16, kind="ExternalInput")
    ident_d = nc.dram_tensor("ident", [P, P], f16, kind="ExternalInput")
    q_d = nc.dram_tensor("q_out", [S, S], f32, kind="ExternalOutput")

    with tile.TileContext(nc) as tc, ExitStack() as ctx:
        w_p = ctx.enter_context(tc.tile_pool(name="w", bufs=1))
        c_p = ctx.enter_context(tc.tile_pool(name="const", bufs=1))
        r_p = ctx.enter_context(tc.tile_pool(name="r", bufs=2))
        x_p = ctx.enter_context(tc.tile_pool(name="x", bufs=2))
        o_p = ctx.enter_context(tc.tile_pool(name="o", bufs=1))
        qa_p = ctx.enter_context(tc.tile_pool(name="qa", bufs=2, space="PSUM"))
        qb_p = ctx.enter_context(tc.tile_pool(name="qb", bufs=2, space="PSUM"))
        t_p = ctx.enter_context(tc.tile_pool(name="t", bufs=1, space="PSUM"))

        # preload the sigmoid activation table while the DMA phase runs
        jnk = c_p.tile([1, 2], f16, tag="jnk")
        nc.vector.memset(jnk[:], 0.0)
        jnk2 = c_p.tile([1, 2], f16, tag="jnk2")
        nc.scalar.activation(jnk2[:], jnk[:], Sig)

        # constants / first-iteration sigmoid operand (host-computed)
        ident_t = c_p.tile([P, P], f16, tag="ident")
        nc.sync.dma_start(ident_t[:], ident_d.ap())
        r1_t = r_p.tile([P, S], f16, tag="r1")
        nc.sync.dma_start(r1_t[:], r01_d.ap())
        # rows 0:64 and 64:128 both hold R2 = sig(s_con)^T rows 128:192
        r2_t = r_p.tile([P, S], f16, tag="r2")
        nc.sync.dma_start(r2_t[:], r02d_d.ap())
        scon1_t = c_p.tile([P, S], f16, tag="scon1")
        nc.gpsimd.dma_start(scon1_t[:], scon1_d.ap())
        scon2_t = c_p.tile([K2, S], f16, tag="scon2")
        nc.gpsimd.dma_start(scon2_t[:], scon2_d.ap())

        # s_bin cache: 3 striped queue tensors, 4 segment chunks each
        wt = [w_p.tile([P, QW], f16, tag=f"wq{q}", name=f"wq{q}") for q in range(3)]
        queues = [nc.sync, nc.scalar, nc.gpsimd]
        for p in range(4):
            sl = slice(p * SEG, (p + 1) * SEG)
            for q in range(3):
                queues[q].dma_start(wt[q][:, sl], wq_d[q].ap()[:, sl])

        def col_matmuls(qa, qb, r1_t, r2_t, j0, j1):
            for j in range(j0, j1):
                q, c1, c2, (p0, p1) = _wslices(j)
                last = j == S - 1
                t = wt[q]
                rj1 = r1_t[:, j:j + 1]
                rj2 = r2_t[p0:p1, j:j + 1]
                nc.tensor.matmul(qa[:, j:j + 1], t[:, c1:c1 + P], rj1,
                                 start=False, stop=False, skip_group_check=True)
                nc.tensor.matmul(qa[:, j:j + 1], t[p0:p1, c2:c2 + P], rj2,
                                 start=False, stop=False, skip_group_check=True)
                nc.tensor.matmul(qb[:, j:j + 1], t[:, c1 + P:c1 + S], rj1,
                                 start=False, stop=last, skip_group_check=True)
                nc.tensor.matmul(qb[:, j:j + 1], t[p0:p1, c2 + P:c2 + S], rj2,
                                 start=False, stop=last, skip_group_check=True)

        CA, CB = P + K2, 2 * P + K2   # tt column bases for t2/t4 blocks

        def boundary_left(qa, qb, x1, x2, tt, r1n):
            nc.scalar.activation(x1[:, 0:P], qa[:, 0:P], Sig)
            nc.scalar.activation(x2[:, 0:P], qb[:, 0:P], Sig)
            nc.tensor.transpose(tt[:, 0:P], x1[:, 0:P], ident_t[:])
            nc.tensor.transpose(tt[:, P:P + K2], x2[:, 0:P], ident_t[0:K2, 0:K2])
            nc.vector.tensor_copy(r1n[:, 0:P], tt[:, 0:P])
            nc.scalar.activation(r1n[:, P:S], tt[:, P:P + K2], Cpy)

        def boundary_right(qa, qb, x1, x2, tt, r2n):
            nc.scalar.activation(x1[:, P:S], qa[:, P:S], Sig)
            nc.scalar.activation(x2[:, P:S], qb[:, P:S], Sig)
            nc.tensor.transpose(tt[0:K2, CA:CA + P], x1[:, P:S], ident_t[:])
            nc.tensor.transpose(tt[K2:P, CA:CA + P], x1[:, P:S], ident_t[:])
            nc.tensor.transpose(tt[0:K2, CB:CB + K2], x2[:, P:S],
                                ident_t[0:K2, 0:K2])
            nc.tensor.transpose(tt[K2:P, CB:CB + K2], x2[:, P:S],
                                ident_t[0:K2, 0:K2])
            # both partition halves of r2 in one copy each
            nc.vector.tensor_copy(r2n[:, 0:P], tt[:, CA:CA + P])
            nc.scalar.activation(r2n[:, P:S], tt[:, CB:CB + K2], Cpy)

        for it in range(3):
            qa = qa_p.tile([P, S], f32, tag="qa")
            qb = qb_p.tile([K2, S], f32, tag="qb")
            # q = s_con first (identity stationary: out[m,c] = rhs[m,c])
            nc.tensor.matmul(qa[:], ident_t[:], scon1_t[:],
                             start=True, stop=False, skip_group_check=True)
            nc.tensor.matmul(qb[:], ident_t[0:K2, 0:K2], scon2_t[:],
                             start=True, stop=False, skip_group_check=True)
            if it < 2:
                x1 = x_p.tile([P, S], f16, tag="x1")
                x2 = x_p.tile([K2, S], f16, tag="x2")
                tt = t_p.tile([P, 3 * P], f16, tag="tt")
                r1n = r_p.tile([P, S], f16, tag="r1")
                r2n = r_p.tile([P, S], f16, tag="r2")
                if it == 0:
                    # DMA-bound: left-half boundary hides under the load
                    col_matmuls(qa, qb, r1_t, r2_t, 0, P)
                    boundary_left(qa, qb, x1, x2, tt, r1n)
                    col_matmuls(qa, qb, r1_t, r2_t, P, S)
                else:
                    col_matmuls(qa, qb, r1_t, r2_t, 0, S)
                    boundary_left(qa, qb, x1, x2, tt, r1n)
                boundary_right(qa, qb, x1, x2, tt, r2n)
                r1_t, r2_t = r1n, r2n
            else:
                o1 = o_p.tile([P, S], f32, tag="o1")
                o2 = o_p.tile([K2, S], f32, tag="o2")
                col_matmuls(qa, qb, r1_t, r2_t, 0, P)
                nc.scalar.activation(o1[:, 0:P], qa[:, 0:P], Sig)
                nc.sync.dma_start(q_d.ap()[0:P, 0:P], o1[:, 0:P])
                nc.scalar.activation(o2[:, 0:P], qb[:, 0:P], Sig)
                nc.gpsimd.dma_start(q_d.ap()[P:S, 0:P], o2[:, 0:P])
                col_matmuls(qa, qb, r1_t, r2_t, P, S)
                nc.scalar.activation(o1[:, P:S], qa[:, P:S], Sig)
                nc.sync.dma_start(q_d.ap()[0:P, P:S], o1[:, P:S])
                nc.scalar.activation(o2[:, P:S], qb[:, P:S], Sig)
                nc.gpsimd.dma_start(q_d.ap()[P:S, P:S], o2[:, P:S])
    nc.compile()
    return nc


def _get_program():
    if "nc" not in _CACHE:
        _CACHE["nc"] = _build_program()
    return _CACHE["nc"]


def _prep_core_inputs(s_con_b, sbm16_b, ident):
    """Per-batch input dict. sbm16_b: masked s_bin, fp16, [i, j, k]."""
    kt = np.ascontiguousarray(sbm16_b.transpose(2, 1, 0))   # [k, j, i]
    w1 = kt[0:P].reshape(P, NB, BW)                          # k 0:128
    w2 = kt[P:S].reshape(K2, NB, BW)                         # k 128:192
    out = {"ident": ident}
    for q in range(3):
        bs = [q + 3 * m for m in range(NB // 3)]
        segs = []
        for p in range(4):
            b0, b1 = bs[2 * p], bs[2 * p + 1]
            segs.append(np.concatenate(
                [w1[:, b0], w1[:, b1],
                 np.concatenate([w2[:, b0], w2[:, b1]], axis=0)], axis=1))
        out[f"wq{q}"] = np.ascontiguousarray(np.concatenate(segs, axis=1))
    sig0 = (1.0 / (1.0 + np.exp(-s_con_b.astype(np.float64)))).astype(np.float16)
    out["r01"] = np.ascontiguousarray(sig0[:, 0:P].T)        # [k 0:128, j]
    r02 = sig0[:, P:S].T                                     # [k 128:192, j]
    out["r02d"] = np.ascontiguousarray(np.concatenate([r02, r02], axis=0))
    sc16 = s_con_b.astype(np.float16)
    out["scon1"] = np.ascontiguousarray(sc16[0:P])
    out["scon2"] = np.ascontiguousarray(sc16[P:S])
    return out


def kernel(s_con, s_bin, mask):
    from concourse.bass_utils import run_bass_kernel_spmd

    s_con = np.asarray(s_con, dtype=np.float32)
    s_bin = np.asarray(s_bin, dtype=np.float32)
    mask = np.asarray(mask)

    idx = np.arange(S)
    ne = idx[:, None] != idx[None, :]                       # [a, k]
    m2 = ne[:, None, :] & ne[None, :, :]                    # [i, j, k]
    full_mask = mask[:, :, :, None] & m2[None]              # [B, i, j, k]
    sbm16 = (s_bin * full_mask).astype(np.float16)

    ident = np.eye(P, dtype=np.float16)
    nc = _get_program()
    in_maps = [_prep_core_inputs(s_con[b], sbm16[b], ident) for b in range(B)]
    res = run_bass_kernel_spmd(nc, in_maps, list(range(B)))
    out = np.stack([res.results[b]["q_out"] for b in range(B)], 0)
    return np.ascontiguousarray(out.astype(np.float32))


# revision 15
# speedup vs baseline: 7.4787x; 1.0288x over previous
"""Trainium2 Bass kernel for MFVIConstituency mean-field iterations.

Per batch b (one NeuronCore each, 8 total):
    q = s_con;  repeat 3x:  q[i,j] = s_con[i,j] + sum_k sig(q)[j,k] * sb[i,j,k]
    out = sigmoid(q)
where sb = s_bin * mask2o, mask2o[i,j,k] = mask[i,j] & (i!=k) & (j!=k).

Formulation: the contraction is a batch of 192 per-j matvecs
    q[:, j] = SB_j @ sig(q)[j, :],   SB_j = sb[:, j, :]  (192x192)
mapped onto the TensorEngine: for each output column j the stationary
operand is sb[k, i; j] (k-tiles 128+64, i-tiles 128+64) and the moving
operand is the single column sig(q)^T[:, j]; 4 matmuls accumulate
q[:, j] in PSUM (fp32).  s_con enters first through identity-stationary
matmuls (start=True sets has_written for the whole tile; a per-column
start would clear has_written BANK-wide and break accumulation).  The
two i-halves of q share one PSUM bank ([128, 384]: rows 0:128 at cols
0:192, rows 128:192 at cols 192:384 on partitions 0:64) so one
activation instruction with a [p, 2, c] access pattern sigmoids both.

s_bin lives in SBUF as fp16.  The DMA cost model charges free-dim bytes
per partition (partition count is free), so everything is packed into
128 partitions: the 64-row k-tile-2 blocks ride the upper partition
half (two j-blocks sharing 128 partitions), and the whole 14.2MB cache
is striped over the three DMA queues (SP / Activation / GpSimd) in
8-column j-blocks, round-robin, so the three transfers overlap and
columns arrive in j order; iteration-1 matmuls stream right behind.
Because lhsT and rhs must share a base partition, sig(q)^T k-rows
128:192 are kept duplicated on partitions 0:64 AND 64:128 (the PE
transposes write each block twice; one DVE copy moves both).

Boundary between iterations builds X = sigmoid(q) (fp16) and R = X^T,
split by column halves so the left half (q cols 0:128) runs while the
right-half matmuls / DMA are still in flight: ACT sigmoid (PSUM->SBUF)
-> PE transpose blocks -> DVE copies.  Output leaves in natural [i, j]
layout.
"""

import numpy as np

S = 192
B = 8
P = 128
K2 = 64          # k-tile-2 rows (k 128:192), also lower half of i
BJ = 8           # j per block
NB = S // BJ     # 24 blocks, striped round-robin over 3 queues
BW = BJ * S      # 1536 elements per (block, k-tile)
SEG = 3 * BW     # 4608 elements per block-pair segment
QW = 4 * SEG     # 18432 elements per queue tensor

_CACHE = {}


def _wslices(j):
    """j -> (queue, w1 col base, w2 col base, w2 partition range)."""
    b, jj = divmod(j, BJ)
    q, m = b % 3, b // 3
    p, which = divmod(m, 2)
    c1 = p * SEG + which * BW + jj * S
    c2 = p * SEG + 2 * BW + jj * S
    pr = (0, K2) if which == 0 else (K2, P)
    return q, c1, c2, pr


def _build_program():
    import concourse.tile as tile
    from concourse import mybir, bacc
    from contextlib import ExitStack

    f32, f16 = mybir.dt.float32, mybir.dt.float16
    Sig = mybir.ActivationFunctionType.Sigmoid

    nc = bacc.Bacc("TRN2", target_bir_lowering=False, debug=False, num_devices=B)
    wq_d = [nc.dram_tensor(f"wq{q}", [P, QW], f16, kind="ExternalInput")
            for q in range(3)]
    scon1_d = nc.dram_tensor("scon1", [P, S], f16, kind="ExternalInput")
    scon2_d = nc.dram_tensor("scon2", [K2, S], f16, kind="ExternalInput")
    r01_d = nc.dram_tensor("r01", [P, S], f16, kind="ExternalInput")
    r02d_d = nc.dram_tensor("r02d", [P, S], f16, kind="ExternalInput")
    ident_d = nc.dram_tensor("ident", [P, P], f16, kind="ExternalInput")
    q_d = nc.dram_tensor("q_out", [S, S], f32, kind="ExternalOutput")

    def lr(ap, lo, hi):
        """[p, 384] tile view -> [p, 2, hi-lo] AP over cols {lo:hi, 192+lo:192+hi}."""
        return ap.rearrange("p (s c) -> p s c", c=S)[:, :, lo:hi]

    with tile.TileContext(nc) as tc, ExitStack() as ctx:
        w_p = ctx.enter_context(tc.tile_pool(name="w", bufs=1))
        c_p = ctx.enter_context(tc.tile_pool(name="const", bufs=1))
        r_p = ctx.enter_context(tc.tile_pool(name="r", bufs=2))
        x_p = ctx.enter_context(tc.tile_pool(name="x", bufs=2))
        o_p = ctx.enter_context(tc.tile_pool(name="o", bufs=1))
        qq_p = ctx.enter_context(tc.tile_pool(name="qq", bufs=2, space="PSUM"))
        t_p = ctx.enter_context(tc.tile_pool(name="t", bufs=1, space="PSUM"))

        # preload the sigmoid activation table while the DMA phase runs
        jnk = c_p.tile([1, 2], f16, tag="jnk")
        nc.vector.memset(jnk[:], 0.0)
        jnk2 = c_p.tile([1, 2], f16, tag="jnk2")
        nc.scalar.activation(jnk2[:], jnk[:], Sig)

        # constants / first-iteration sigmoid operand (host-computed)
        ident_t = c_p.tile([P, P], f16, tag="ident")
        nc.sync.dma_start(ident_t[:], ident_d.ap())
        r1_t = r_p.tile([P, S], f16, tag="r1")
        nc.sync.dma_start(r1_t[:], r01_d.ap())
        # rows 0:64 and 64:128 both hold R2 = sig(s_con)^T rows 128:192
        r2_t = r_p.tile([P, S], f16, tag="r2")
        nc.sync.dma_start(r2_t[:], r02d_d.ap())
        scon1_t = c_p.tile([P, S], f16, tag="scon1")
        nc.gpsimd.dma_start(scon1_t[:], scon1_d.ap())
        scon2_t = c_p.tile([K2, S], f16, tag="scon2")
        nc.gpsimd.dma_start(scon2_t[:], scon2_d.ap())

        # s_bin cache: 3 striped queue tensors, 4 segment chunks each
        wt = [w_p.tile([P, QW], f16, tag=f"wq{q}", name=f"wq{q}") for q in range(3)]
        queues = [nc.sync, nc.scalar, nc.gpsimd]
        for p in range(4):
            sl = slice(p * SEG, (p + 1) * SEG)
            for q in range(3):
                queues[q].dma_start(wt[q][:, sl], wq_d[q].ap()[:, sl])

        def col_matmuls(qq, r1_t, r2_t, j0, j1):
            for j in range(j0, j1):
                q, c1, c2, (p0, p1) = _wslices(j)
                last = j == S - 1
                t = wt[q]
                rj1 = r1_t[:, j:j + 1]
                rj2 = r2_t[p0:p1, j:j + 1]
                nc.tensor.matmul(qq[:, j:j + 1], t[:, c1:c1 + P], rj1,
                                 start=False, stop=False, skip_group_check=True)
                nc.tensor.matmul(qq[:, j:j + 1], t[p0:p1, c2:c2 + P], rj2,
                                 start=False, stop=False, skip_group_check=True)
                nc.tensor.matmul(qq[0:K2, S + j:S + j + 1],
                                 t[:, c1 + P:c1 + S], rj1,
                                 start=False, stop=last, skip_group_check=True)
                nc.tensor.matmul(qq[0:K2, S + j:S + j + 1],
                                 t[p0:p1, c2 + P:c2 + S], rj2,
                                 start=False, stop=last, skip_group_check=True)

        CA, CB = P + K2, 2 * P + K2   # tt column bases for t2/t4 blocks

        def boundary_left(qq, xx, tt, r1n):
            nc.scalar.activation(lr(xx[:], 0, P), lr(qq[:], 0, P), Sig)
            nc.tensor.transpose(tt[:, 0:P], xx[:, 0:P], ident_t[:])
            nc.tensor.transpose(tt[:, P:P + K2], xx[0:K2, S:S + P],
                                ident_t[0:K2, 0:K2])
            nc.vector.tensor_copy(r1n[:, 0:P], tt[:, 0:P])
            nc.vector.tensor_copy(r1n[:, P:S], tt[:, P:P + K2])

        def boundary_right(qq, xx, tt, r2n):
            nc.scalar.activation(lr(xx[:], P, S), lr(qq[:], P, S), Sig)
            nc.tensor.transpose(tt[0:K2, CA:CA + P], xx[:, P:S], ident_t[:])
            nc.tensor.transpose(tt[K2:P, CA:CA + P], xx[:, P:S], ident_t[:])
            nc.tensor.transpose(tt[0:K2, CB:CB + K2], xx[0:K2, S + P:2 * S],
                                ident_t[0:K2, 0:K2])
            nc.tensor.transpose(tt[K2:P, CB:CB + K2], xx[0:K2, S + P:2 * S],
                                ident_t[0:K2, 0:K2])
            # both partition halves of r2 in one copy each
            nc.vector.tensor_copy(r2n[:, 0:P], tt[:, CA:CA + P])
            nc.vector.tensor_copy(r2n[:, P:S], tt[:, CB:CB + K2])

        for it in range(3):
            qq = qq_p.tile([P, 2 * S], f32, tag="qq")
            # q = s_con first (identity stationary: out[m,c] = rhs[m,c]).
            # The second matmul spans all 128 partitions (zeros on 64:128)
            # so the combined sigmoid below reads only written PSUM.
            nc.tensor.matmul(qq[:, 0:S], ident_t[:], scon1_t[:],
                             start=True, stop=False, skip_group_check=True)
            nc.tensor.matmul(qq[:, S:2 * S], ident_t[0:K2, :], scon2_t[:],
                             start=False, stop=False, skip_group_check=True)
            if it < 2:
                xx = x_p.tile([P, 2 * S], f16, tag="xx")
                tt = t_p.tile([P, 3 * P], f16, tag="tt")
                r1n = r_p.tile([P, S], f16, tag="r1")
                r2n = r_p.tile([P, S], f16, tag="r2")
                if it == 0:
                    # DMA-bound: left-half boundary hides under the load
                    col_matmuls(qq, r1_t, r2_t, 0, P)
                    boundary_left(qq, xx, tt, r1n)
                    col_matmuls(qq, r1_t, r2_t, P, S)
                else:
                    col_matmuls(qq, r1_t, r2_t, 0, S)
                    boundary_left(qq, xx, tt, r1n)
                boundary_right(qq, xx, tt, r2n)
                r1_t, r2_t = r1n, r2n
            else:
                oo = o_p.tile([P, 2 * S], f32, tag="oo")
                col_matmuls(qq, r1_t, r2_t, 0, P)
                nc.scalar.activation(lr(oo[:], 0, P), lr(qq[:], 0, P), Sig)
                nc.sync.dma_start(q_d.ap()[0:P, 0:P], oo[:, 0:P])
                nc.gpsimd.dma_start(q_d.ap()[P:S, 0:P], oo[0:K2, S:S + P])
                col_matmuls(qq, r1_t, r2_t, P, S)
                nc.scalar.activation(lr(oo[:], P, S), lr(qq[:], P, S), Sig)
                nc.sync.dma_start(q_d.ap()[0:P, P:S], oo[:, P:S])
                nc.gpsimd.dma_start(q_d.ap()[P:S, P:S], oo[0:K2, S + P:2 * S])
    nc.compile()
    return nc


def _get_program():
    if "nc" not in _CACHE:
        _CACHE["nc"] = _build_program()
    return _CACHE["nc"]


def _prep_core_inputs(s_con_b, sbm16_b, ident):
    """Per-batch input dict. sbm16_b: masked s_bin, fp16, [i, j, k]."""
    kt = np.ascontiguousarray(sbm16_b.transpose(2, 1, 0))   # [k, j, i]
    w1 = kt[0:P].reshape(P, NB, BW)                          # k 0:128
    w2 = kt[P:S].reshape(K2, NB, BW)                         # k 128:192
    out = {"ident": ident}
    for q in range(3):
        bs = [q + 3 * m for m in range(NB // 3)]
        segs = []
        for p in range(4):
            b0, b1 = bs[2 * p], bs[2 * p + 1]
            segs.append(np.concatenate(
                [w1[:, b0], w1[:, b1],
                 np.concatenate([w2[:, b0], w2[:, b1]], axis=0)], axis=1))
        out[f"wq{q}"] = np.ascontiguousarray(np.concatenate(segs, axis=1))
    sig0 = (1.0 / (1.0 + np.exp(-s_con_b.astype(np.float64)))).astype(np.float16)
    out["r01"] = np.ascontiguousarray(sig0[:, 0:P].T)        # [k 0:128, j]
    r02 = sig0[:, P:S].T                                     # [k 128:192, j]
    out["r02d"] = np.ascontiguousarray(np.concatenate([r02, r02], axis=0))
    sc16 = s_con_b.astype(np.float16)
    out["scon1"] = np.ascontiguousarray(sc16[0:P])
    out["scon2"] = np.ascontiguousarray(sc16[P:S])
    return out


def kernel(s_con, s_bin, mask):
    from concourse.bass_utils import run_bass_kernel_spmd

    s_con = np.asarray(s_con, dtype=np.float32)
    s_bin = np.asarray(s_bin, dtype=np.float32)
    mask = np.asarray(mask)

    idx = np.arange(S)
    ne = idx[:, None] != idx[None, :]                       # [a, k]
    m2 = ne[:, None, :] & ne[None, :, :]                    # [i, j, k]
    full_mask = mask[:, :, :, None] & m2[None]              # [B, i, j, k]
    sbm16 = (s_bin * full_mask).astype(np.float16)

    ident = np.eye(P, dtype=np.float16)
    nc = _get_program()
    in_maps = [_prep_core_inputs(s_con[b], sbm16[b], ident) for b in range(B)]
    res = run_bass_kernel_spmd(nc, in_maps, list(range(B)))
    out = np.stack([res.results[b]["q_out"] for b in range(B)], 0)
    return np.ascontiguousarray(out.astype(np.float32))
